# revision 1
# baseline (speedup 1.0000x reference)
import numpy as np

N = 50000
E = 800000
G = 250
F = 128
H = 128
DOUT = 32
NC = 8


def _sharded_impl(nodes, senders, receivers, n_node, is_root_mask,
                  W0, b0, W1, b1, Wg, bg, devs):
    import jax
    import jax.numpy as jnp
    from functools import partial

    nodes = np.asarray(nodes, np.float32)
    senders = np.asarray(senders, np.int32)
    receivers = np.asarray(receivers, np.int32)
    n_node = np.asarray(n_node, np.int32)
    is_root_mask = np.asarray(is_root_mask, np.float32)

    self_idx = np.arange(N, dtype=np.int32)
    s = np.concatenate([senders, self_idx])
    r = np.concatenate([receivers, self_idx])
    etot = s.shape[0]
    pad = (-etot) % NC
    w = np.ones(etot, np.float32)
    if pad:
        s = np.concatenate([s, np.zeros(pad, np.int32)])
        r = np.concatenate([r, np.zeros(pad, np.int32)])
        w = np.concatenate([w, np.zeros(pad, np.float32)])
    per = s.shape[0] // NC
    s_sh = s.reshape(NC, per)
    r_sh = r.reshape(NC, per)
    w_sh = w.reshape(NC, per)
    gi = np.repeat(np.arange(G, dtype=np.int32), n_node)
    if gi.shape[0] != N:
        gi = np.resize(gi, N)

    @partial(jax.pmap, axis_name="x", devices=devs)
    def run(s_l, r_l, w_l, nd, mask, gidx, w0, bb0, w1, bb1, wg, bbg):
        # layer 0: local gather + local segment_sum over this core's edge
        # slice, then cross-core reduction of the partial aggregates
        part0 = jax.ops.segment_sum(nd[s_l] * w_l[:, None], r_l,
                                    num_segments=N)
        agg0 = jax.lax.psum(part0, "x")
        h = jax.nn.relu(agg0 @ w0 + bb0)
        feats = jnp.concatenate([h, nd], axis=-1)
        part1 = jax.ops.segment_sum(feats[s_l] * w_l[:, None], r_l,
                                    num_segments=N)
        agg1 = jax.lax.psum(part1, "x")
        h1 = jax.nn.relu(agg1 @ w1 + bb1)
        masked = h1 * mask[:, None]
        hg = jax.ops.segment_sum(masked, gidx, num_segments=G)
        return hg @ wg + bbg

    def rep(a):
        a = np.asarray(a)
        return np.ascontiguousarray(np.broadcast_to(a, (NC,) + a.shape))

    out = run(s_sh, r_sh, w_sh, rep(nodes), rep(is_root_mask), rep(gi),
              rep(W0), rep(b0), rep(W1), rep(b1), rep(Wg), rep(bg))
    return np.asarray(out[0], dtype=np.float32)


def _cpu_impl(nodes, senders, receivers, n_node, is_root_mask,
              W0, b0, W1, b1, Wg, bg):
    import jax
    import jax.numpy as jnp

    cpu = jax.devices("cpu")[0]
    with jax.default_device(cpu):
        nodes = jnp.asarray(nodes)
        self_idx = jnp.arange(N, dtype=jnp.int32)
        s = jnp.concatenate([jnp.asarray(senders, jnp.int32), self_idx])
        r = jnp.concatenate([jnp.asarray(receivers, jnp.int32), self_idx])
        agg0 = jax.ops.segment_sum(nodes[s], r, num_segments=N)
        h = jax.nn.relu(agg0 @ jnp.asarray(W0) + jnp.asarray(b0))
        feats = jnp.concatenate([h, nodes], axis=-1)
        agg1 = jax.ops.segment_sum(feats[s], r, num_segments=N)
        h = jax.nn.relu(agg1 @ jnp.asarray(W1) + jnp.asarray(b1))
        masked = h * jnp.asarray(is_root_mask)[:, None]
        gi = jnp.repeat(jnp.arange(G), jnp.asarray(n_node),
                        total_repeat_length=N)
        hg = jax.ops.segment_sum(masked, gi, num_segments=G)
        out = hg @ jnp.asarray(Wg) + jnp.asarray(bg)
        return np.asarray(out, dtype=np.float32)


def kernel(**inputs):
    try:
        import jax
        devs = [d for d in jax.devices() if d.platform != "cpu"][:NC]
        if len(devs) == NC:
            return _sharded_impl(devs=devs, **inputs)
    except Exception:
        pass
    return _cpu_impl(**inputs)

